# revision 35
# baseline (speedup 1.0000x reference)
"""Causal self-attention kernel for Trainium2, 8 NeuronCores.

Sharding: DP4 x TP2. Core c = 2*b + g handles batch b (2048 tokens) and
head-group g (8 of 16 heads). Per core:
  - x arrives pre-cast to bf16 token-major; the PE transposes it tile by
    tile against a host-provided identity (no DMA xbar transpose, so all
    startup DMAs run concurrently),
  - QKV matmuls in bf16: Q,K dim-major ([head_dim, tokens]), V token-major
    with a ones column at col 64 (softmax denominator for free),
  - attention per head pair with one-cgroup software lookahead: scores^T =
    K_h^T-tile @ Q_h in [k, q] layout, both heads' QK matmuls in different
    PE row groups, one wide exp on ACT (1/sqrt(64) folded into its scale)
    into bf16 probs, causal handling by emitting only needed column ranges
    (no memsets) and a 0/1 mask multiply on the 128-wide diagonal band;
    the AV matmul streams only the live column range,
  - normalization: reciprocal_approx_fast in place on the PSUM ones-row,
    gpsimd partition_broadcast, DVE scale into bf16 dim-major yT,
  - bf16 projection with the w_proj row shard, pairwise AllReduce
    (cores 2b, 2b+1) writing straight into the output DRAM tensor,
    512-token chunks except the last q-tile which drains in 128-token
    chunks to shrink the tail.

QKV+transpose work for token tile n+1 and projection/AllReduce for tile
n-1 are zippered into the attention unit stream for tile n so the PE
always has independent matmuls while ACT drains the exps.

Everything (shapes, sharding) is hardcoded for
x: [4, 2048, 1024], w_qkv: [1024, 3072], w_proj: [1024, 1024], f32.
"""

import ml_dtypes
import numpy as np

import concourse.bacc as bacc
import concourse.mybir as mybir
import concourse.tile as tile
from concourse.bass_utils import run_bass_kernel_spmd

F32 = mybir.dt.float32
BF16 = mybir.dt.bfloat16
FP8 = mybir.dt.float8e4
WP_SCALE = 64.0  # host-side w_proj scale so fp8e4 stays in normal range

S = 2048  # tokens per core (one batch element)
D = 1024  # d_model
HL = 8  # heads per core (local)
HD = 64  # head dim
GD = HL * HD  # 512, head-group dim
NQT = S // 512  # 4 q-tiles of 512
NDM = D // 128  # 8 d_model chunks
NTOK = S // 128  # 16 token tiles of 128
VP = 128  # per-head V row: 64 v cols + ones col + zero pad

_NC_CACHE = {}


class _Ctx:
    pass


def _x_dma_units(nc, P, n):
    """DMA the 4 token tiles of q-tile n into the x_tm ring."""
    units = []
    for t4 in range(4):
        t = 4 * n + t4

        def emit(t=t):
            xt = P.x_p.tile([128, D], BF16, tag="xtm", name="xtm")
            P.x_tm[t] = xt
            nc.sync.dma_start(out=xt, in_=P.xb16[t * 128 : (t + 1) * 128, :])

        units.append(emit)
    return units


def _tr_units(nc, P, n):
    """PE-transpose x token tiles 4n..4n+3 into xT[:, k, n*512:(n+1)*512]."""
    units = []
    for k in range(NDM):

        def emit(k=k):
            ps = P.b1_ps.tile([128, 1024], BF16, tag="b1", name="trps")
            for t4 in range(4):
                t = 4 * n + t4
                nc.tensor.transpose(
                    ps[:, t4 * 128 : (t4 + 1) * 128],
                    P.x_tm[t][:, k * 128 : (k + 1) * 128],
                    P.ident,
                )
            nc.vector.tensor_copy(
                out=P.xT[:, k, n * 512 : (n + 1) * 512], in_=ps[:, 0:512]
            )

        units.append(emit)
    return units


def _qkv_units(nc, P, n):
    """QKV matmul chains for token tile n, as separately emittable units."""
    units = []

    def qk_chain(m):
        def emit():
            ps = P.b1_ps.tile([128, 512], F32, tag="b1", name="qkps")
            for k in range(NDM):
                nc.tensor.matmul(
                    ps,
                    P.w_sb[:, k, m * 128 : (m + 1) * 128],
                    P.xT[:, k, n * 512 : (n + 1) * 512],
                    start=(k == 0),
                    stop=(k == NDM - 1),
                )
            nc.vector.tensor_copy(
                out=P.qkT[:, m, n * 512 : (n + 1) * 512], in_=ps
            )

        return emit

    def v_chain(t4):
        def emit():
            t = n * 4 + t4
            ps = P.b1_ps.tile([128, 512], F32, tag="b1", name="vps")
            for k in range(NDM):
                nc.tensor.matmul(
                    ps,
                    P.xT[:, k, t * 128 : (t + 1) * 128],
                    P.w_sb[:, k, 2 * GD : 3 * GD],
                    start=(k == 0),
                    stop=(k == NDM - 1),
                )
            nc.scalar.copy(
                out=P.v_sb[:, t, :, 0:HD],
                in_=ps.rearrange("p (h d) -> p h d", h=HL),
            )

        return emit

    for m in range(2 * GD // 128):
        units.append(qk_chain(m))
    for t4 in range(4):
        units.append(v_chain(t4))
    return units


def _attn_units(nc, P, j):
    """Attention units for q-tile j with one-cgroup lookahead per head pair.

    Per head pair: [alloc, se(0), se(1), ma(0), se(2), ma(1), ...,
    ma(C-1), epilogue] where se = scores+exp, ma = mask+AV. The gap
    between se(i+1) and ma(i) is where zipped units land, keeping the PE
    fed while ACT computes exp(i+1).
    """
    units = []
    NC_ = 4 * j + 4
    for hp in range(HL // 2):
        yps = {}
        probs = {}

        def alloc(hp=hp, yps=yps):
            for hi in range(2):
                yps[hi] = P.y_ps.tile(
                    [128, 512], F32, tag=f"yps{hi}", name=f"yps{hi}", bufs=1
                )

        units.append(alloc)

        def score_exp(c, hp=hp, yps=yps, probs=probs):
            def emit():
                d = c - 4 * j  # >= 0 on the diagonal band
                off = max(d, 0) * 128  # columns below off are fully masked
                sps2 = P.attn_ps.tile(
                    [128, 2, 512], F32, tag="sps2", name="sps2"
                )
                for hi in range(2):
                    h = 2 * hp + hi
                    po = (h % 2) * 64
                    nc.tensor.matmul(
                        sps2[:, hi, off:512],
                        P.qkT[po : po + 64, 4 + h // 2, c * 128 : (c + 1) * 128],
                        P.qkT[po : po + 64, h // 2, j * 512 + off : (j + 1) * 512],
                        start=True,
                        stop=True,
                    )
                probs2 = P.probs_p.tile(
                    [128, 2, 512], BF16, tag="probs", name="probs"
                )
                probs[c] = probs2
                nc.scalar.activation(
                    out=probs2[:, :, off:512],
                    in_=sps2[:, :, off:512],
                    func=mybir.ActivationFunctionType.Exp,
                    scale=0.125,
                )

            return emit

        def mask_av(c, hp=hp, yps=yps, probs=probs):
            def emit():
                d = c - 4 * j
                off = max(d, 0) * 128
                probs2 = probs.pop(c)
                if d >= 0:
                    for hi in range(2):
                        nc.vector.tensor_mul(
                            probs2[:, hi, off : off + 128],
                            probs2[:, hi, off : off + 128],
                            P.mask_sb,
                        )
                for hi in range(2):
                    h = 2 * hp + hi
                    nc.tensor.matmul(
                        yps[hi][:, off:512],
                        P.v_sb[:, c, h, :],
                        probs2[:, hi, off:512],
                        start=(c == 0),
                        stop=(c == NC_ - 1),
                    )

            return emit
        def epilogue(hp=hp, yps=yps):
            # ones-row out of PSUM, fast reciprocal, partition broadcast,
            # scale y into dim-major bf16 yT; the two heads' chains are
            # interleaved so both yps banks free as early as possible
            dens = {}
            denbs = {}
            for hi in range(2):
                den = P.den_p.tile([1, 512], F32, tag="den", name="den")
                nc.vector.tensor_copy(out=den, in_=yps[hi][HD : HD + 1, :])
                nc.vector.reciprocal_approx_fast(out=den, in_=den)
                dens[hi] = den
            for hi in range(2):
                denb = P.den_p.tile([HD, 512], F32, tag="denb", name="denb")
                nc.gpsimd.partition_broadcast(denb, dens[hi])
                denbs[hi] = denb
            for hi in range(2):
                h = 2 * hp + hi
                po = (h % 2) * 64
                nc.vector.tensor_mul(
                    P.yT[po : po + 64, h // 2, j * 512 : (j + 1) * 512],
                    yps[hi][0:HD, :],
                    denbs[hi],
                )

        se = [score_exp(c) for c in range(NC_)]
        ma = [mask_av(c) for c in range(NC_)]
        units.append(se[0])
        for c in range(1, NC_):
            units.append(se[c])
            units.append(ma[c - 1])
        units.append(ma[NC_ - 1])
        units.append(epilogue)
    return units


def _proj_units(nc, P, mt):
    """Projection for token tile mt (128 tokens, token-major output)."""

    def emit():
        osb = P.out_p.tile([128, D], F32, tag="osb", name="osb")
        for nh in range(2):
            ps = P.b1_ps.tile([128, 512], F32, tag="b1", name="ops")
            for kk in range(GD // 128):
                nc.tensor.matmul(
                    ps,
                    P.yT[:, kk, mt * 128 : (mt + 1) * 128],
                    P.wp_sb[:, kk, nh * 512 : (nh + 1) * 512],
                    start=(kk == 0),
                    stop=(kk == GD // 128 - 1),
                )
            nc.vector.tensor_copy(out=osb[:, nh * 512 : (nh + 1) * 512], in_=ps)
        nc.sync.dma_start(out=P.cc_in[mt * 128 : (mt + 1) * 128, :], in_=osb)

    return [emit]


def _ar_unit(nc, P, lo, hi):
    """AllReduce token rows [lo, hi), then DMA them to the output."""

    def emit():
        nc.gpsimd.collective_compute(
            "AllReduce",
            mybir.AluOpType.add,
            replica_groups=[[0, 1], [2, 3], [4, 5], [6, 7]],
            ins=[P.cc_in[lo:hi, :].opt()],
            outs=[P.cc_out[lo:hi, :].opt()],
        )
        for mt in range(lo // 128, hi // 128):
            nc.sync.dma_start(
                out=P.out[mt * 128 : (mt + 1) * 128, :],
                in_=P.cc_out[mt * 128 : (mt + 1) * 128, :],
            )

    return [emit]


def _zip_emit(a_units, z_units):
    """Emit a_units with z_units spread evenly between them."""
    zi = 0
    la = max(len(a_units), 1)
    for i, u in enumerate(a_units):
        u()
        while zi < len(z_units) and zi * la < (i + 1) * len(z_units):
            z_units[zi]()
            zi += 1
    for u in z_units[zi:]:
        u()


def _build_nc():
    nc = bacc.Bacc(None, num_devices=8)
    P = _Ctx()

    P.xb16 = nc.dram_tensor("xb16", [S, D], BF16, kind="ExternalInput").ap()
    wqkv = nc.dram_tensor("wqkv", [D, 3 * GD], BF16, kind="ExternalInput").ap()
    wproj = nc.dram_tensor("wproj", [GD, D], BF16, kind="ExternalInput").ap()
    wprojf = nc.dram_tensor("wprojf", [D, D], BF16, kind="ExternalInput").ap()
    consts = nc.dram_tensor("consts", [128, 256], BF16, kind="ExternalInput").ap()
    P.out = nc.dram_tensor("out", [S, D], F32, kind="ExternalOutput").ap()

    with tile.TileContext(nc) as tc:
        with (
            tc.tile_pool(name="const", bufs=1) as const,
            tc.tile_pool(name="x_p", bufs=6) as x_p,
            tc.tile_pool(name="w_p", bufs=1) as w_p,
            tc.tile_pool(name="big_p", bufs=1) as big_p,
            tc.tile_pool(name="probs_p", bufs=5) as probs_p,
            tc.tile_pool(name="den_p", bufs=2) as den_p,
            tc.tile_pool(name="out_p", bufs=2) as out_p,
            tc.tile_pool(name="b1_ps", bufs=2, space="PSUM") as b1_ps,
            tc.tile_pool(name="attn_ps", bufs=2, space="PSUM") as attn_ps,
            tc.tile_pool(name="y_ps", bufs=1, space="PSUM") as y_ps,
            tc.tile_pool(name="dram", bufs=1, space="DRAM") as dram,
        ):
            P.x_p, P.probs_p, P.den_p, P.out_p = x_p, probs_p, den_p, out_p
            P.b1_ps, P.attn_ps, P.y_ps = b1_ps, attn_ps, y_ps
            P.x_tm = {}

            cb = const.tile([128, 256], BF16, name="cb")
            nc.sync.dma_start(out=cb, in_=consts)
            P.mask_sb = cb[:, 0:128]
            P.ident = cb[:, 128:256]

            # weights: per-column-block DMAs so early QKV chains unblock
            # before the whole tensor lands
            P.w_sb = w_p.tile([128, NDM, 3 * GD], BF16, name="w_sb")

            def w_dma():
                for m in range(3 * GD // 128):
                    nc.sync.dma_start(
                        out=P.w_sb[:, :, m * 128 : (m + 1) * 128],
                        in_=wqkv[:, m * 128 : (m + 1) * 128].rearrange(
                            "(k p) c -> p k c", p=128
                        ),
                    )

            P.w_dma = w_dma
            P.wp_sb = w_p.tile([128, GD // 128, D], BF16, name="wp_sb")
            P.wpf_sb = w_p.tile([128, 2, GD // 128, D], BF16, name="wpf_sb")

            def wp_dma():
                for kk in range(GD // 128):
                    nc.sync.dma_start(
                        out=P.wp_sb[:, kk, :],
                        in_=wproj[kk * 128 : (kk + 1) * 128, :],
                    )
                for pg in range(2):
                    for kk in range(GD // 128):
                        r = pg * GD + kk * 128
                        nc.sync.dma_start(
                            out=P.wpf_sb[:, pg, kk, :],
                            in_=wprojf[r : r + 128, :],
                        )

            P.wp_dma = wp_dma

            P.xT = big_p.tile([128, NDM, S], BF16, name="xT")
            P.qkT = big_p.tile([128, 2 * GD // 128, S], BF16, name="qkT")
            P.v_sb = big_p.tile([128, NTOK, HL, VP], BF16, name="v_sb")
            nc.gpsimd.memset(P.v_sb[:, :, :, HD:VP], 0.0)
            nc.gpsimd.memset(P.v_sb[:, :, :, HD : HD + 1], 1.0)
            P.yT = big_p.tile([128, GD // 128, S], BF16, name="yT")

            P.cc_in = dram.tile([S, D], F32, name="cc_in")
            P.cc_out = dram.tile([S, D], F32, name="cc_out")
            P.y_in = dram.tile([128, GD // 128, 512], BF16, name="y_in")
            P.y_out = dram.tile([2, 128, GD // 128, 512], BF16, name="y_out")
            P.yG = big_p.tile([128, 2, GD // 128, 512], BF16, name="yG")

            # PE p-state prewarm: keep the array busy while the first x
            # tiles stream in, so transposes/QKV start at the warm clock
            warm = const.tile([128, 128], BF16, name="warm")
            nc.gpsimd.memset(warm, 1.0)
            wsink = const.tile([1, 1], F32, name="wsink")
            wps = P.b1_ps.tile([128, 512], F32, tag="b1", name="wps")
            for _ in range(48):
                nc.tensor.matmul(wps[:, 0:128], warm, warm, start=True, stop=True)
            nc.vector.tensor_copy(out=wsink, in_=wps[0:1, 0:1])

            # startup wave 1: x tiles 0-3 + all of wqkv, then transposes(0);
            # wave 2 (x 4-15, wproj) queues up behind wave 1 on the DMA rings
            for u in _x_dma_units(nc, P, 0):
                u()
            P.w_dma()
            for u in _tr_units(nc, P, 0):
                u()
            for u in (
                _x_dma_units(nc, P, 1)
                + _x_dma_units(nc, P, 2)
                + _x_dma_units(nc, P, 3)
            ):
                u()
            P.wp_dma()
            for u in _qkv_units(nc, P, 0):
                u()

            for n in range(NQT):
                a_units = _attn_units(nc, P, n)
                z = []
                if n >= 1:
                    for mt in range(4 * (n - 1), 4 * n):
                        z += _proj_units(nc, P, mt)
                    z += _ar_unit(nc, P, (n - 1) * 512, n * 512)
                if n + 1 < NQT:
                    z += _tr_units(nc, P, n + 1)
                    z += _qkv_units(nc, P, n + 1)
                _zip_emit(a_units, z)

            # tail: AllGather the tiny fp8 y of the last q-tile within each
            # pair, then every core computes the full projection locally and
            # writes the output directly — 8x less collective payload than
            # AllReducing the f32 projection output
            nc.sync.dma_start(out=P.y_in, in_=P.yT[:, :, 3 * 512 : 4 * 512])
            nc.gpsimd.collective_compute(
                "AllGather",
                mybir.AluOpType.bypass,
                replica_groups=[[0, 1], [2, 3], [4, 5], [6, 7]],
                ins=[P.y_in.opt()],
                outs=[P.y_out.opt()],
            )
            nc.sync.dma_start(
                out=P.yG, in_=P.y_out.rearrange("g p k t -> p g k t")
            )
            for mt in range(4 * (NQT - 1), 4 * NQT):
                tt = mt - 4 * (NQT - 1)
                osb = P.out_p.tile([128, D], F32, tag="osb", name="osb")
                for nh in range(2):
                    ps = P.b1_ps.tile([128, 512], F32, tag="b1", name="ops")
                    i = 0
                    for pg in range(2):
                        for kk in range(GD // 128):
                            nc.tensor.matmul(
                                ps,
                                P.yG[:, pg, kk, tt * 128 : (tt + 1) * 128],
                                P.wpf_sb[:, pg, kk, nh * 512 : (nh + 1) * 512],
                                start=(i == 0),
                                stop=(i == 7),
                            )
                            i += 1
                    nc.vector.tensor_copy(
                        out=osb[:, nh * 512 : (nh + 1) * 512], in_=ps
                    )
                nc.sync.dma_start(
                    out=P.out[mt * 128 : (mt + 1) * 128, :], in_=osb
                )

    nc.compile()
    return nc


def _host_consts():
    ki = np.arange(128)[:, None]
    qj = np.arange(128)[None, :]
    consts = np.zeros((128, 256), dtype=ml_dtypes.bfloat16)
    consts[:, 0:128] = (qj >= ki).astype(ml_dtypes.bfloat16)  # causal band
    consts[:, 128:256] = np.eye(128, dtype=ml_dtypes.bfloat16)
    return consts


def _in_maps(x, w_qkv, w_proj):
    consts = _host_consts()
    maps = []
    for c in range(8):
        b, g = c // 2, c % 2
        wq = w_qkv[:, g * GD : (g + 1) * GD]
        wk = w_qkv[:, D + g * GD : D + (g + 1) * GD]
        wv = w_qkv[:, 2 * D + g * GD : 2 * D + (g + 1) * GD]
        maps.append(
            {
                "xb16": np.ascontiguousarray(x[b]).astype(ml_dtypes.bfloat16),
                "wqkv": np.ascontiguousarray(
                    np.concatenate([wq, wk, wv], axis=1)
                ).astype(ml_dtypes.bfloat16),
                "wproj": np.ascontiguousarray(
                    w_proj[g * GD : (g + 1) * GD, :]
                ).astype(ml_dtypes.bfloat16),
                "wprojf": np.ascontiguousarray(w_proj).astype(
                    ml_dtypes.bfloat16
                ),
                "consts": consts,
            }
        )
    return maps


def kernel(x, w_qkv, w_proj):
    x = np.ascontiguousarray(x, dtype=np.float32)
    w_qkv = np.ascontiguousarray(w_qkv, dtype=np.float32)
    w_proj = np.ascontiguousarray(w_proj, dtype=np.float32)
    if "nc" not in _NC_CACHE:
        _NC_CACHE["nc"] = _build_nc()
    nc = _NC_CACHE["nc"]
    r = run_bass_kernel_spmd(nc, _in_maps(x, w_qkv, w_proj), list(range(8)))
    return np.stack([r.results[2 * b]["out"] for b in range(4)], axis=0)


# revision 38
# speedup vs baseline: 1.0508x; 1.0508x over previous
"""Causal self-attention kernel for Trainium2, 8 NeuronCores.

Sharding: DP4 x TP2. Core c = 2*b + g handles batch b (2048 tokens) and
head-group g (8 of 16 heads). Per core:
  - x arrives pre-cast to bf16 token-major; the PE transposes it tile by
    tile against a host-provided identity (no DMA xbar transpose, so all
    startup DMAs run concurrently),
  - QKV matmuls in bf16: Q,K dim-major ([head_dim, tokens]), V token-major
    with a ones column at col 64 (softmax denominator for free),
  - attention per head pair with one-cgroup software lookahead: scores^T =
    K_h^T-tile @ Q_h in [k, q] layout, both heads' QK matmuls in different
    PE row groups, one wide exp on ACT (1/sqrt(64) folded into its scale)
    into bf16 probs, causal handling by emitting only needed column ranges
    (no memsets) and a 0/1 mask multiply on the 128-wide diagonal band;
    the AV matmul streams only the live column range,
  - normalization: reciprocal_approx_fast in place on the PSUM ones-row,
    gpsimd partition_broadcast, DVE scale into bf16 dim-major yT,
  - bf16 projection with the w_proj row shard, pairwise AllReduce
    (cores 2b, 2b+1) writing straight into the output DRAM tensor,
    512-token chunks except the last q-tile which drains in 128-token
    chunks to shrink the tail.

QKV+transpose work for token tile n+1 and projection/AllReduce for tile
n-1 are zippered into the attention unit stream for tile n so the PE
always has independent matmuls while ACT drains the exps.

Everything (shapes, sharding) is hardcoded for
x: [4, 2048, 1024], w_qkv: [1024, 3072], w_proj: [1024, 1024], f32.
"""

import ml_dtypes
import numpy as np

import concourse.bacc as bacc
import concourse.mybir as mybir
import concourse.tile as tile
from concourse.bass_utils import run_bass_kernel_spmd

F32 = mybir.dt.float32
BF16 = mybir.dt.bfloat16
FP8 = mybir.dt.float8e4
WP_SCALE = 64.0  # host-side w_proj scale so fp8e4 stays in normal range

S = 2048  # tokens per core (one batch element)
D = 1024  # d_model
HL = 8  # heads per core (local)
HD = 64  # head dim
GD = HL * HD  # 512, head-group dim
NQT = S // 512  # 4 q-tiles of 512
NDM = D // 128  # 8 d_model chunks
NTOK = S // 128  # 16 token tiles of 128
VP = 128  # per-head V row: 64 v cols + ones col + zero pad

_NC_CACHE = {}


class _Ctx:
    pass


def _x_dma_units(nc, P, n):
    """DMA the 4 token tiles of q-tile n into the x_tm ring."""
    units = []
    for t4 in range(4):
        t = 4 * n + t4

        def emit(t=t):
            xt = P.x_p.tile([128, D], BF16, tag="xtm", name="xtm")
            P.x_tm[t] = xt
            nc.sync.dma_start(out=xt, in_=P.xb16[t * 128 : (t + 1) * 128, :])

        units.append(emit)
    return units


def _tr_units(nc, P, n):
    """PE-transpose x token tiles 4n..4n+3 into xT[:, k, n*512:(n+1)*512]."""
    units = []
    for k in range(NDM):

        def emit(k=k):
            ps = P.b1_ps.tile([128, 1024], BF16, tag="b1", name="trps")
            for t4 in range(4):
                t = 4 * n + t4
                nc.tensor.transpose(
                    ps[:, t4 * 128 : (t4 + 1) * 128],
                    P.x_tm[t][:, k * 128 : (k + 1) * 128],
                    P.ident,
                )
            nc.vector.tensor_copy(
                out=P.xT[:, k, n * 512 : (n + 1) * 512], in_=ps[:, 0:512]
            )

        units.append(emit)
    return units


def _qkv_units(nc, P, n):
    """QKV matmul chains for token tile n, as separately emittable units."""
    units = []

    def qk_chain(m):
        def emit():
            ps = P.b1_ps.tile([128, 512], F32, tag="b1", name="qkps")
            for k in range(NDM):
                nc.tensor.matmul(
                    ps,
                    P.w_sb[:, k, m * 128 : (m + 1) * 128],
                    P.xT[:, k, n * 512 : (n + 1) * 512],
                    start=(k == 0),
                    stop=(k == NDM - 1),
                )
            nc.vector.tensor_copy(
                out=P.qkT[:, m, n * 512 : (n + 1) * 512], in_=ps
            )

        return emit

    def v_chain(t4):
        def emit():
            t = n * 4 + t4
            ps = P.b1_ps.tile([128, 512], F32, tag="b1", name="vps")
            for k in range(NDM):
                nc.tensor.matmul(
                    ps,
                    P.xT[:, k, t * 128 : (t + 1) * 128],
                    P.w_sb[:, k, 2 * GD : 3 * GD],
                    start=(k == 0),
                    stop=(k == NDM - 1),
                )
            nc.scalar.copy(
                out=P.v_sb[:, t, :, 0:HD],
                in_=ps.rearrange("p (h d) -> p h d", h=HL),
            )

        return emit

    for m in range(2 * GD // 128):
        units.append(qk_chain(m))
    for t4 in range(4):
        units.append(v_chain(t4))
    return units


def _attn_units(nc, P, j):
    """Attention units for q-tile j with one-cgroup lookahead per head pair.

    Per head pair: [alloc, se(0), se(1), ma(0), se(2), ma(1), ...,
    ma(C-1), epilogue] where se = scores+exp, ma = mask+AV. The gap
    between se(i+1) and ma(i) is where zipped units land, keeping the PE
    fed while ACT computes exp(i+1).
    """
    units = []
    NC_ = 4 * j + 4
    for hp in range(HL // 2):
        yps = {}
        probs = {}

        def alloc(hp=hp, yps=yps):
            for hi in range(2):
                yps[hi] = P.y_ps.tile(
                    [128, 512], F32, tag=f"yps{hi}", name=f"yps{hi}", bufs=1
                )

        units.append(alloc)

        def score_exp(c, hp=hp, yps=yps, probs=probs):
            def emit():
                d = c - 4 * j  # >= 0 on the diagonal band
                off = max(d, 0) * 128  # columns below off are fully masked
                sps2 = P.attn_ps.tile(
                    [128, 2, 512], F32, tag="sps2", name="sps2"
                )
                for hi in range(2):
                    h = 2 * hp + hi
                    po = (h % 2) * 64
                    nc.tensor.matmul(
                        sps2[:, hi, off:512],
                        P.qkT[po : po + 64, 4 + h // 2, c * 128 : (c + 1) * 128],
                        P.qkT[po : po + 64, h // 2, j * 512 + off : (j + 1) * 512],
                        start=True,
                        stop=True,
                    )
                probs2 = P.probs_p.tile(
                    [128, 2, 512], BF16, tag="probs", name="probs"
                )
                probs[c] = probs2
                nc.scalar.activation(
                    out=probs2[:, :, off:512],
                    in_=sps2[:, :, off:512],
                    func=mybir.ActivationFunctionType.Exp,
                    scale=0.125,
                )

            return emit

        def mask_av(c, hp=hp, yps=yps, probs=probs):
            def emit():
                d = c - 4 * j
                off = max(d, 0) * 128
                probs2 = probs.pop(c)
                if d >= 0:
                    for hi in range(2):
                        nc.vector.tensor_mul(
                            probs2[:, hi, off : off + 128],
                            probs2[:, hi, off : off + 128],
                            P.mask_sb,
                        )
                for hi in range(2):
                    h = 2 * hp + hi
                    nc.tensor.matmul(
                        yps[hi][:, off:512],
                        P.v_sb[:, c, h, :],
                        probs2[:, hi, off:512],
                        start=(c == 0),
                        stop=(c == NC_ - 1),
                    )

            return emit
        def epilogue(hp=hp, yps=yps):
            # ones-row out of PSUM, fast reciprocal, partition broadcast,
            # scale y into dim-major bf16 yT
            for hi in range(2):
                h = 2 * hp + hi
                po = (h % 2) * 64
                den = P.den_p.tile([1, 512], F32, tag="den", name="den")
                nc.vector.tensor_copy(out=den, in_=yps[hi][HD : HD + 1, :])
                nc.vector.reciprocal_approx_fast(out=den, in_=den)
                denb = P.den_p.tile([HD, 512], F32, tag="denb", name="denb")
                nc.gpsimd.partition_broadcast(denb, den)
                nc.vector.tensor_mul(
                    P.yT[po : po + 64, h // 2, j * 512 : (j + 1) * 512],
                    yps[hi][0:HD, :],
                    denb,
                )

        se = [score_exp(c) for c in range(NC_)]
        ma = [mask_av(c) for c in range(NC_)]
        units.append(se[0])
        for c in range(1, NC_):
            units.append(se[c])
            units.append(ma[c - 1])
        units.append(ma[NC_ - 1])
        units.append(epilogue)
    return units


def _proj_units(nc, P, mt):
    """Projection for token tile mt (128 tokens, token-major output)."""

    def emit():
        osb = P.out_p.tile([128, D], F32, tag="osb", name="osb")
        for nh in range(2):
            ps = P.b1_ps.tile([128, 512], F32, tag="b1", name="ops")
            for kk in range(GD // 128):
                nc.tensor.matmul(
                    ps,
                    P.yT[:, kk, mt * 128 : (mt + 1) * 128],
                    P.wp_sb[:, kk, nh * 512 : (nh + 1) * 512],
                    start=(kk == 0),
                    stop=(kk == GD // 128 - 1),
                )
            nc.vector.tensor_copy(out=osb[:, nh * 512 : (nh + 1) * 512], in_=ps)
        nc.sync.dma_start(out=P.cc_in[mt * 128 : (mt + 1) * 128, :], in_=osb)

    return [emit]


def _ar_unit(nc, P, lo, hi):
    """AllReduce token rows [lo, hi), then DMA them to the output."""

    def emit():
        nc.gpsimd.collective_compute(
            "AllReduce",
            mybir.AluOpType.add,
            replica_groups=[[0, 1], [2, 3], [4, 5], [6, 7]],
            ins=[P.cc_in[lo:hi, :].opt()],
            outs=[P.cc_out[lo:hi, :].opt()],
        )
        for mt in range(lo // 128, hi // 128):
            nc.sync.dma_start(
                out=P.out[mt * 128 : (mt + 1) * 128, :],
                in_=P.cc_out[mt * 128 : (mt + 1) * 128, :],
            )

    return [emit]


def _zip_emit(a_units, z_units):
    """Emit a_units with z_units spread evenly between them."""
    zi = 0
    la = max(len(a_units), 1)
    for i, u in enumerate(a_units):
        u()
        while zi < len(z_units) and zi * la < (i + 1) * len(z_units):
            z_units[zi]()
            zi += 1
    for u in z_units[zi:]:
        u()


def _build_nc():
    nc = bacc.Bacc(None, num_devices=8)
    P = _Ctx()

    P.xb16 = nc.dram_tensor("xb16", [S, D], BF16, kind="ExternalInput").ap()
    wqkv = nc.dram_tensor("wqkv", [D, 3 * GD], BF16, kind="ExternalInput").ap()
    wproj = nc.dram_tensor("wproj", [GD, D], BF16, kind="ExternalInput").ap()
    wprojf = nc.dram_tensor("wprojf", [D, D], BF16, kind="ExternalInput").ap()
    consts = nc.dram_tensor("consts", [128, 256], BF16, kind="ExternalInput").ap()
    P.out = nc.dram_tensor("out", [S, D], F32, kind="ExternalOutput").ap()

    with tile.TileContext(nc) as tc:
        with (
            tc.tile_pool(name="const", bufs=1) as const,
            tc.tile_pool(name="x_p", bufs=6) as x_p,
            tc.tile_pool(name="w_p", bufs=1) as w_p,
            tc.tile_pool(name="big_p", bufs=1) as big_p,
            tc.tile_pool(name="probs_p", bufs=5) as probs_p,
            tc.tile_pool(name="den_p", bufs=2) as den_p,
            tc.tile_pool(name="out_p", bufs=2) as out_p,
            tc.tile_pool(name="b1_ps", bufs=2, space="PSUM") as b1_ps,
            tc.tile_pool(name="attn_ps", bufs=2, space="PSUM") as attn_ps,
            tc.tile_pool(name="y_ps", bufs=1, space="PSUM") as y_ps,
            tc.tile_pool(name="dram", bufs=1, space="DRAM") as dram,
        ):
            P.x_p, P.probs_p, P.den_p, P.out_p = x_p, probs_p, den_p, out_p
            P.b1_ps, P.attn_ps, P.y_ps = b1_ps, attn_ps, y_ps
            P.x_tm = {}

            cb = const.tile([128, 256], BF16, name="cb")
            nc.sync.dma_start(out=cb, in_=consts)
            P.mask_sb = cb[:, 0:128]
            P.ident = cb[:, 128:256]

            # weights: per-column-block DMAs so early QKV chains unblock
            # before the whole tensor lands
            P.w_sb = w_p.tile([128, NDM, 3 * GD], BF16, name="w_sb")

            def w_dma():
                for m in range(3 * GD // 128):
                    nc.sync.dma_start(
                        out=P.w_sb[:, :, m * 128 : (m + 1) * 128],
                        in_=wqkv[:, m * 128 : (m + 1) * 128].rearrange(
                            "(k p) c -> p k c", p=128
                        ),
                    )

            P.w_dma = w_dma
            P.wp_sb = w_p.tile([128, GD // 128, D], BF16, name="wp_sb")
            P.wpf_sb = w_p.tile([128, 2, GD // 128, D], BF16, name="wpf_sb")

            def wp_dma():
                for kk in range(GD // 128):
                    nc.sync.dma_start(
                        out=P.wp_sb[:, kk, :],
                        in_=wproj[kk * 128 : (kk + 1) * 128, :],
                    )
                for pg in range(2):
                    for kk in range(GD // 128):
                        r = pg * GD + kk * 128
                        nc.sync.dma_start(
                            out=P.wpf_sb[:, pg, kk, :],
                            in_=wprojf[r : r + 128, :],
                        )

            P.wp_dma = wp_dma

            P.xT = big_p.tile([128, NDM, S], BF16, name="xT")
            P.qkT = big_p.tile([128, 2 * GD // 128, S], BF16, name="qkT")
            P.v_sb = big_p.tile([128, NTOK, HL, VP], BF16, name="v_sb")
            nc.gpsimd.memset(P.v_sb[:, :, :, HD:VP], 0.0)
            nc.gpsimd.memset(P.v_sb[:, :, :, HD : HD + 1], 1.0)
            P.yT = big_p.tile([128, GD // 128, S], BF16, name="yT")

            P.cc_in = dram.tile([S, D], F32, name="cc_in")
            P.cc_out = dram.tile([S, D], F32, name="cc_out")
            P.y_in = dram.tile([128, GD // 128, 512], BF16, name="y_in")
            P.y_out = dram.tile([2, 128, GD // 128, 512], BF16, name="y_out")
            P.yG = big_p.tile([128, 2, GD // 128, 512], BF16, name="yG")

            # startup wave 1: x tiles 0-3 + all of wqkv, then transposes(0);
            # wave 2 (x 4-15, wproj) queues up behind wave 1 on the DMA rings
            for u in _x_dma_units(nc, P, 0):
                u()
            P.w_dma()
            for u in _tr_units(nc, P, 0):
                u()
            for u in (
                _x_dma_units(nc, P, 1)
                + _x_dma_units(nc, P, 2)
                + _x_dma_units(nc, P, 3)
            ):
                u()
            P.wp_dma()
            for u in _qkv_units(nc, P, 0):
                u()

            for n in range(NQT):
                a_units = _attn_units(nc, P, n)
                z = []
                if n >= 1:
                    for mt in range(4 * (n - 1), 4 * n):
                        z += _proj_units(nc, P, mt)
                    z += _ar_unit(nc, P, (n - 1) * 512, n * 512)
                if n + 1 < NQT:
                    z += _tr_units(nc, P, n + 1)
                    z += _qkv_units(nc, P, n + 1)
                _zip_emit(a_units, z)

            # tail: AllGather the tiny fp8 y of the last q-tile within each
            # pair, then every core computes the full projection locally and
            # writes the output directly — 8x less collective payload than
            # AllReducing the f32 projection output
            # stage y per kk-block: blocks of already-finished head pairs
            # DMA out while the remaining pairs are still computing
            for kk in range(GD // 128):
                nc.sync.dma_start(
                    out=P.y_in[:, kk, :],
                    in_=P.yT[:, kk, 3 * 512 : 4 * 512],
                )
            nc.gpsimd.collective_compute(
                "AllGather",
                mybir.AluOpType.bypass,
                replica_groups=[[0, 1], [2, 3], [4, 5], [6, 7]],
                ins=[P.y_in.opt()],
                outs=[P.y_out.opt()],
            )
            # read the gathered y back split by pair half on separate
            # DMA queues
            for pg in range(2):
                nc.sync.dma_start(
                    out=P.yG[:, pg, :, :],
                    in_=P.y_out[pg].rearrange("p k t -> p (k t)"),
                )
            for mt in range(4 * (NQT - 1), 4 * NQT):
                tt = mt - 4 * (NQT - 1)
                osb = P.out_p.tile([128, D], F32, tag="osb", name="osb")
                for nh in range(2):
                    ps = P.b1_ps.tile([128, 512], F32, tag="b1", name="ops")
                    i = 0
                    for pg in range(2):
                        for kk in range(GD // 128):
                            nc.tensor.matmul(
                                ps,
                                P.yG[:, pg, kk, tt * 128 : (tt + 1) * 128],
                                P.wpf_sb[:, pg, kk, nh * 512 : (nh + 1) * 512],
                                start=(i == 0),
                                stop=(i == 7),
                            )
                            i += 1
                    nc.vector.tensor_copy(
                        out=osb[:, nh * 512 : (nh + 1) * 512], in_=ps
                    )
                nc.sync.dma_start(
                    out=P.out[mt * 128 : (mt + 1) * 128, :], in_=osb
                )

    nc.compile()
    return nc


def _host_consts():
    ki = np.arange(128)[:, None]
    qj = np.arange(128)[None, :]
    consts = np.zeros((128, 256), dtype=ml_dtypes.bfloat16)
    consts[:, 0:128] = (qj >= ki).astype(ml_dtypes.bfloat16)  # causal band
    consts[:, 128:256] = np.eye(128, dtype=ml_dtypes.bfloat16)
    return consts


def _in_maps(x, w_qkv, w_proj):
    consts = _host_consts()
    maps = []
    for c in range(8):
        b, g = c // 2, c % 2
        wq = w_qkv[:, g * GD : (g + 1) * GD]
        wk = w_qkv[:, D + g * GD : D + (g + 1) * GD]
        wv = w_qkv[:, 2 * D + g * GD : 2 * D + (g + 1) * GD]
        maps.append(
            {
                "xb16": np.ascontiguousarray(x[b]).astype(ml_dtypes.bfloat16),
                "wqkv": np.ascontiguousarray(
                    np.concatenate([wq, wk, wv], axis=1)
                ).astype(ml_dtypes.bfloat16),
                "wproj": np.ascontiguousarray(
                    w_proj[g * GD : (g + 1) * GD, :]
                ).astype(ml_dtypes.bfloat16),
                "wprojf": np.ascontiguousarray(w_proj).astype(
                    ml_dtypes.bfloat16
                ),
                "consts": consts,
            }
        )
    return maps


def kernel(x, w_qkv, w_proj):
    x = np.ascontiguousarray(x, dtype=np.float32)
    w_qkv = np.ascontiguousarray(w_qkv, dtype=np.float32)
    w_proj = np.ascontiguousarray(w_proj, dtype=np.float32)
    if "nc" not in _NC_CACHE:
        _NC_CACHE["nc"] = _build_nc()
    nc = _NC_CACHE["nc"]
    r = run_bass_kernel_spmd(nc, _in_maps(x, w_qkv, w_proj), list(range(8)))
    return np.stack([r.results[2 * b]["out"] for b in range(4)], axis=0)
